# revision 23
# baseline (speedup 1.0000x reference)
"""CoNE KG-embedding scoring kernel for 8 Trainium2 NeuronCores.

Strategy (v2): entity-sharded SBUF-resident fp8 tables + GPSIMD ap_gather.

The two big gathers (nei_embed[neiMatrix[src]] and ent_embed[dst]) are the
memory-bound core of this problem.  indirect DMA runs ~139ns/row on one
queue; instead we shard both tables into 64 = 8 cores x 8 (16-partition
groups) buckets of EPB=1563 entities.  Partition 16g+i of core c holds
d-slice [16i,16i+16) of bucket (c,g)'s entities, fp8.  A single ap_gather
instruction then gathers, per group, an independent per-group list of rows
(~30ns/row, 8 Q7 cores in parallel).

Each (b,n) / (b,k) pair is routed on host to the bucket owning its entity.
Phase 1 (neighbor attention) accumulates unnormalized softmax numerators
v[b] = sum_k exp(q.nkv/16) * nkv and l[b] = sum_k exp(.) in a fixed layout
of C1=3 slots per (group, b) (overflow pairs are relocated to other groups,
with the entity row appended to that group's table extension), reduces
per-b on DVE, combines groups on PE, and AllReduces across cores.
t = (1-w)*ie + rel + w*v/l is formed directly in the gather ("tilde")
layout: q~ and base~ = (1-w)*ie + rel are host-precomputed uploads.
Phase 2 gathers ent rows and t rows per pair and reduces |t - pe| on
DVE + PE (block-ones matmul) into per-(group, slot) scores; the host
scatters them back to [B, N] and negates.
"""

import numpy as np
import ml_dtypes

import concourse.bacc as bacc
import concourse.bass as bass
import concourse.mybir as mybir
import concourse.tile as tile
from concourse.bass_utils import run_bass_kernel_spmd

F32 = mybir.dt.float32
BF16 = mybir.dt.bfloat16
FP8 = mybir.dt.float8e4
I16 = mybir.dt.int16
NP_FP8 = ml_dtypes.float8_e4m3
NP_BF16 = ml_dtypes.bfloat16

E, R, D, K, B, N = 100000, 500, 256, 64, 1024, 256
NCORES = 8
NG = 8                 # 16-partition groups per core
NBK = NCORES * NG      # 64 entity buckets
EPB = 1563             # entities per bucket (64*1563 = 100032 >= E)
TE = EPB               # ent table rows per group
EXT1 = 192             # nei table extension rows (relocated entities)
TN = EPB + EXT1
C1 = 3                 # phase-1 slots per (group, b)
L1 = C1 * B            # 3072 phase-1 slots per group
CH1 = 384              # phase-1 chunk (128 b's)
L2 = 4608              # phase-2 slots per group (mean 4096, +8 sigma)
CH2 = 384              # phase-2 chunk
MCOL = 384             # matmul free-dim chunk (psum-bank safe)
MASK_OFF = -1.0e9
INV_SQRT_D = 1.0 / 16.0

_PROGRAMS = {}
LAST_RESULT = None


def _build_program(iters=1, skip_ph1=False, skip_ph2=False, skip_ar=False, dbl_g2=False, ph2_nocompute=False):
    nc = bacc.Bacc(
        "TRN2",
        target_bir_lowering=False,
        debug=False,
        enable_asserts=False,
        num_devices=NCORES,
    )

    def din(name, shape, dt):
        return nc.dram_tensor(name, shape, dt, kind="ExternalInput").ap()

    ent_t = din("ent_tbl", [128, TE, 16], FP8)
    nei_t = din("nei_tbl", [128, TN, 16], FP8)
    qt_t = din("qt", [128, B, 16], FP8)
    base_t = din("base", [128, B, 16], FP8)
    wrep_t = din("wrep", [128, B], BF16)
    nkv_i = din("nkv_idx", [128, L1 // 16], I16)
    ent_i = din("ent_idx", [128, L2 // 16], I16)
    t_i = din("t_idx", [128, L2 // 16], I16)
    offs_t = din("offs1", [8, L1], F32)
    w8_t = din("w8", [128, 8], BF16)
    w2_t = din("w2", [8, 128], BF16)
    wc_t = din("wc", [128, 16], BF16)
    out_t = nc.dram_tensor("out", [8, L2], F32, kind="ExternalOutput").ap()

    ar_in = [
        nc.dram_tensor(f"ar_in{it}", [16, B * 18], BF16, kind="Internal").ap()
        for it in range(iters)
    ]
    ar_out = [
        nc.dram_tensor(f"ar_out{it}", [16, B * 18], BF16, kind="Internal",
                       addr_space="Shared").ap()
        for it in range(iters)
    ]
    groups = [list(range(NCORES))]

    with tile.TileContext(nc) as tc:
      with (
          tc.tile_pool(name="persist", bufs=1) as pp,
          tc.tile_pool(name="pent", bufs=5) as p2e,
          tc.psum_pool(name="ps", bufs=2) as psp,
      ):
        for it in range(iters):
            # ---------- persistent small loads ---------------------------
            w8 = pp.tile([128, 8], BF16, tag="w8")
            nc.sync.dma_start(out=w8[:], in_=w8_t[:])
            w2 = pp.tile([8, 128], BF16, tag="w2")
            nc.sync.dma_start(out=w2[:], in_=w2_t[:])
            wc = pp.tile([128, 16], BF16, tag="wc")
            nc.sync.dma_start(out=wc[:], in_=wc_t[:])
            vt = pp.tile([128, B, 18], BF16, tag="vt")
            tt = pp.tile([128, B, 16], FP8, tag="tt")
            ent_sb = pp.tile([128, TE, 16], FP8, tag="ent")
            nc.sync.dma_start(out=ent_sb[:], in_=ent_t[:])
            ent_idx = pp.tile([128, L2 // 16], I16, tag="enti")
            nc.sync.dma_start(out=ent_idx[:], in_=ent_i[:])
            ent_gs = {}

            def gather_ent(ch):
                csl_ = slice(ch * (CH2 // 16), (ch + 1) * (CH2 // 16))
                eg = p2e.tile([128, CH2, 16], FP8, tag="eg")
                for _r in range(2 if dbl_g2 else 1):
                    nc.gpsimd.ap_gather(eg[:], ent_sb[:],
                                        ent_idx[:, csl_], 128, TE, 16, CH2)
                ent_gs[ch] = eg

            # ---------- phase 1a: sharded neighbor attention -------------
            with tc.tile_pool(name="ph1g", bufs=2) as p1g, \
                 tc.tile_pool(name="ph1d", bufs=2) as p1d, \
                 tc.tile_pool(name="ph1a", bufs=1) as p1a:
                nei_sb = p1a.tile([128, TN, 16], FP8, tag="nei")
                nc.sync.dma_start(out=nei_sb[:], in_=nei_t[:])
                qt = p1a.tile([128, B, 16], FP8, tag="qt")
                nc.sync.dma_start(out=qt[:], in_=qt_t[:])
                nkv_idx = p1a.tile([128, L1 // 16], I16, tag="nkvi")
                nc.sync.dma_start(out=nkv_idx[:], in_=nkv_i[:])
                offs = p1a.tile([8, L1], F32, tag="offs")
                nc.sync.dma_start(out=offs[:], in_=offs_t[:])

                if skip_ph1:
                    nc.vector.memset(vt[:], 1.0)
                for ch in range(0 if skip_ph1 else L1 // CH1):
                    csl = slice(ch * (CH1 // 16), (ch + 1) * (CH1 // 16))
                    nkv_g = p1g.tile([128, CH1, 16], FP8, tag="nkv")
                    nc.gpsimd.ap_gather(nkv_g[:], nei_sb[:],
                                        nkv_idx[:, csl], 128, TN, 16, CH1)
                    # scores: per-partition partial dot, then group-sum on PE
                    # q side needs no gather: slot = b*C1 + c, so q~[b] is a
                    # stride-0 broadcast over the C1 slot repeats.
                    bsl0 = slice(ch * (CH1 // C1), (ch + 1) * (CH1 // C1))
                    q_bc = (qt[:, bsl0, None, :]
                            .to_broadcast([128, CH1 // C1, C1, 16]))
                    prod = p1d.tile([128, CH1, 18], BF16, tag="prod")
                    sprod = prod[:, :, 0:16]
                    nc.vector.tensor_mul(
                        out=sprod.rearrange("p (b c) j -> p b c j", c=C1),
                        in0=nkv_g[:].rearrange("p (b c) j -> p b c j", c=C1),
                        in1=q_bc)
                    sred = p1d.tile([128, CH1], BF16, tag="sred")
                    with nc.allow_low_precision(reason="16-term bf16 sum"):
                        nc.vector.tensor_reduce(
                            out=sred[:], in_=sprod, axis=mybir.AxisListType.X,
                            op=mybir.AluOpType.add)
                    p_sb = p1d.tile([8, CH1], BF16, tag="p")
                    for m in range(CH1 // MCOL):
                        msl = slice(m * MCOL, (m + 1) * MCOL)
                        ps = psp.tile([8, MCOL], F32, tag="sc")
                        nc.tensor.matmul(ps[:], w8[:], sred[:, msl],
                                         start=True, stop=True)
                        sc2 = p1d.tile([8, MCOL], F32, tag="sc2")
                        nc.vector.tensor_add(
                            out=sc2[:], in0=ps[:],
                            in1=offs[:, ch * CH1 + m * MCOL:
                                     ch * CH1 + (m + 1) * MCOL])
                        nc.scalar.activation(
                            out=p_sb[:, msl], in_=sc2[:],
                            func=mybir.ActivationFunctionType.Exp,
                            scale=INV_SQRT_D)
                    # broadcast p to all 16 partitions of each group
                    pb = p1d.tile([128, CH1], F32, tag="pb")
                    for m in range(CH1 // MCOL):
                        msl = slice(m * MCOL, (m + 1) * MCOL)
                        psb = psp.tile([128, MCOL], F32, tag="pb_ps")
                        nc.tensor.matmul(psb[:], w2[:], p_sb[:, msl],
                                         start=True, stop=True)
                        nc.vector.tensor_copy(out=pb[:, msl], in_=psb[:])
                    nc.vector.tensor_mul(
                        out=prod[:, :, 0:16], in0=nkv_g[:],
                        in1=pb[:, :, None].to_broadcast([128, CH1, 16]))
                    nc.vector.tensor_copy(
                        out=prod[:, :, 16:18],
                        in_=pb[:, :, None].to_broadcast([128, CH1, 2]))
                    # segment-reduce the C1 slots per b directly into vt
                    bsl = slice(ch * (CH1 // C1), (ch + 1) * (CH1 // C1))
                    with nc.allow_low_precision(reason="3-term bf16 sum"):
                        nc.vector.tensor_reduce(
                            out=vt[:, bsl, :],
                            in_=prod[:].rearrange("p (b c) j -> p b j c", c=C1),
                            axis=mybir.AxisListType.X,
                            op=mybir.AluOpType.add)

            # prefetch phase-2 ent-side gathers; they run on the Q7 while
            # the DVE/PE finish phase 1 and the AllReduce is in flight
            PREF = 4
            NCH2 = 0 if skip_ph2 else L2 // CH2
            for ch in range(min(PREF, NCH2)):
                gather_ent(ch)

            # ---------- phase 1b: combine groups, AllReduce, build t~ ----
            with tc.tile_pool(name="ph1b", bufs=1) as p1b:
                stage = p1b.tile([16, B * 18], BF16, tag="stage")
                vt_flat = vt[:].rearrange("p b j -> p (b j)")
                for m in range((B * 18) // MCOL):
                    msl = slice(m * MCOL, (m + 1) * MCOL)
                    psc = psp.tile([16, MCOL], F32, tag="vc")
                    nc.tensor.matmul(psc[:], wc[:], vt_flat[:, msl],
                                     start=True, stop=True)
                    nc.vector.tensor_copy(out=stage[:, msl], in_=psc[:])
                nc.sync.dma_start(out=ar_in[it][:], in_=stage[:])
                if not skip_ar:
                    nc.gpsimd.collective_compute(
                        kind="AllReduce", op=mybir.AluOpType.add,
                        replica_groups=groups, ins=[ar_in[it][:]],
                        outs=[ar_out[it][:]])
                else:
                    nc.sync.dma_start(out=ar_out[it][:], in_=ar_in[it][:])
                # load the reduced [16, B*18] into group 0, cast to bf16,
                # then replicate to the other 7 groups
                vall = p1b.tile([128, B, 18], BF16, tag="vall")
                vall_f = vall[:].rearrange("p b j -> p (b j)")
                nc.sync.dma_start(out=vall_f[0:16, :], in_=ar_out[it][:])
                for g in range(1, NG):
                    nc.sync.dma_start(
                        out=vall_f[g * 16:(g + 1) * 16, :], in_=vall_f[0:16, :])
                # t~ = base + wrep * v/l   (all in gather layout)
                wrep = p1b.tile([128, B], BF16, tag="wrep")
                nc.sync.dma_start(out=wrep[:], in_=wrep_t[:])
                base_sb = p1b.tile([128, B, 16], FP8, tag="base")
                nc.sync.dma_start(out=base_sb[:], in_=base_t[:])
                with nc.allow_low_precision(reason="elementwise, not accum"):
                    nc.vector.reciprocal(out=vall[:, :, 17],
                                         in_=vall[:, :, 16])
                    nc.vector.tensor_mul(
                        out=vall[:, :, 0:16], in0=vall[:, :, 0:16],
                        in1=vall[:, :, 17:18].to_broadcast([128, B, 16]))
                    nc.vector.tensor_mul(
                        out=vall[:, :, 0:16], in0=vall[:, :, 0:16],
                        in1=wrep[:, :, None].to_broadcast([128, B, 16]))
                    nc.vector.tensor_add(out=tt[:], in0=vall[:, :, 0:16],
                                         in1=base_sb[:])

            # ---------- phase 2: TransE-L1 scores ------------------------
            with tc.tile_pool(name="ph2g", bufs=2) as p2g, \
                 tc.tile_pool(name="ph2d", bufs=2) as p2d, \
                 tc.tile_pool(name="ph2c", bufs=1) as p2c:
                t_idx = p2c.tile([128, L2 // 16], I16, tag="ti")
                nc.sync.dma_start(out=t_idx[:], in_=t_i[:])
                scores = p2c.tile([8, L2], F32, tag="scores")
                if (ph2_nocompute or skip_ph2):
                    nc.vector.memset(scores[:], 0.0)
                for ch in range(NCH2):
                    csl = slice(ch * (CH2 // 16), (ch + 1) * (CH2 // 16))
                    t_g = p2g.tile([128, CH2, 16], FP8, tag="tg")
                    for _r in range(2 if dbl_g2 else 1):
                        nc.gpsimd.ap_gather(t_g[:], tt[:],
                                            t_idx[:, csl], 128, B, 16, CH2)
                    if ch + PREF < NCH2:
                        gather_ent(ch + PREF)
                    ent_g = ent_gs.pop(ch)
                    if ph2_nocompute:
                        continue
                    # |a-b| = max(a,b) - min(a,b), all on DVE (no engine hops)
                    df = p2d.tile([128, CH2, 16], BF16, tag="df")
                    mn = p2d.tile([128, CH2, 16], BF16, tag="mn")
                    nc.vector.tensor_tensor(out=df[:], in0=ent_g[:],
                                            in1=t_g[:],
                                            op=mybir.AluOpType.max)
                    nc.vector.tensor_tensor(out=mn[:], in0=ent_g[:],
                                            in1=t_g[:],
                                            op=mybir.AluOpType.min)
                    nc.vector.tensor_sub(out=df[:], in0=df[:], in1=mn[:])
                    dred = p2d.tile([128, CH2], BF16, tag="dred")
                    with nc.allow_low_precision(reason="16-term bf16 sum"):
                        nc.vector.tensor_reduce(
                            out=dred[:], in_=df[:], axis=mybir.AxisListType.X,
                            op=mybir.AluOpType.add)
                    for m in range(CH2 // MCOL):
                        msl = slice(m * MCOL, (m + 1) * MCOL)
                        ps2 = psp.tile([8, MCOL], F32, tag="sc2b")
                        nc.tensor.matmul(ps2[:], w8[:], dred[:, msl],
                                         start=True, stop=True)
                        nc.vector.tensor_copy(
                            out=scores[:, ch * CH2 + m * MCOL:
                                       ch * CH2 + (m + 1) * MCOL],
                            in_=ps2[:])
                nc.sync.dma_start(out=out_t[:], in_=scores[:])

    nc.compile()
    return nc


def _get_program(iters=1):
    if iters not in _PROGRAMS:
        _PROGRAMS[iters] = _build_program(iters)
    return _PROGRAMS[iters]


# ---------------------------------------------------------------------------
# host-side preparation
# ---------------------------------------------------------------------------

def _interleave(x):
    """[B, 256] -> [128, B, 16] tilde layout: out[16g+i, b, j] = x[b, 16i+j]."""
    b = x.shape[0]
    base = x.reshape(b, 16, 16).transpose(1, 0, 2)          # [i, b, j]
    return np.tile(base, (NG, 1, 1))                        # [128, b, 16]


def _pack_idx(lists):
    """[NG, L] int16 -> wrapped [128, L//16]: arr[16g+p, c] = lists[g][c*16+p]."""
    ng, L = lists.shape
    return (lists.reshape(ng, L // 16, 16)
            .transpose(0, 2, 1)
            .reshape(ng * 16, L // 16)
            .astype(np.int16))


def make_in_maps(src, rel, dst, ent_embed, rel_embed, nei_embed, weight_embed,
                 neiMatrix):
    rng_fail = []

    ent8 = np.zeros((NBK * EPB, D), dtype=NP_FP8)
    ent8[:E] = ent_embed.astype(NP_FP8)
    nei8 = np.zeros((NBK * EPB, D), dtype=NP_FP8)
    nei8[:E] = nei_embed.astype(NP_FP8)

    # d16 bucketed tables: [64, EPB, 16i, 16j] -> per core [8g,16i,EPB,16j]
    ent_bkt = ent8.reshape(NBK, EPB, 16, 16)
    nei_bkt = nei8.reshape(NBK, EPB, 16, 16)

    ie = ent_embed[src]                          # [B, D] f32
    rel_e = rel_embed[rel]                       # [B, D]
    w = 1.0 / (1.0 + np.exp(-weight_embed[src].reshape(B, 1)))  # sigmoid
    q = ie + rel_e
    base = (1.0 - w) * ie + rel_e

    qt_il = _interleave(q.astype(NP_FP8))                    # [128, B, 16] fp8
    base_il = _interleave(base.astype(NP_FP8))               # [128, B, 16] fp8
    wrep = np.tile(w.reshape(1, B), (128, 1)).astype(NP_BF16)

    w8 = np.zeros((128, 8), dtype=NP_BF16)
    w8[np.arange(128), np.arange(128) // 16] = 1.0
    w2 = np.zeros((8, 128), dtype=NP_BF16)
    w2[np.arange(128) // 16, np.arange(128)] = 1.0
    wc = np.zeros((128, 16), dtype=NP_BF16)
    wc[np.arange(128), np.arange(128) % 16] = 1.0

    # ---------------- phase-1 routing (per core) -------------------------
    nei_ids = neiMatrix[src]                     # [B, K]
    valid = nei_ids > 0
    bb, kk = np.nonzero(valid)
    ee = nei_ids[bb, kk]
    bkt = ee // EPB
    core1 = bkt // NG
    grp1 = bkt % NG
    loc1 = ee - bkt * EPB

    nkv_idx = np.zeros((NCORES, NG, L1), dtype=np.int64)
    q1_idx = np.zeros((NCORES, NG, L1), dtype=np.int64)
    offs1 = np.full((NCORES, NG, L1), MASK_OFF, dtype=np.float32)
    ext_maps = [dict() for _ in range(NCORES * NG)]   # (c,g) -> {e: extpos}
    nei_ext = np.zeros((NCORES, NG, EXT1, D), dtype=NP_FP8)

    for c in range(NCORES):
        m = core1 == c
        cb, cg, cl, ce = bb[m], grp1[m], loc1[m], ee[m]
        # rank within (g, b)
        order = np.lexsort((cb, cg))
        cb, cg, cl, ce = cb[order], cg[order], cl[order], ce[order]
        grp_key = cg * B + cb
        # cumcount per key
        uniq, starts = np.unique(grp_key, return_index=True)
        ranks = np.arange(len(grp_key))
        ranks = ranks - np.repeat(starts, np.diff(np.append(starts,
                                                            len(grp_key))))
        nat = ranks < C1
        slots = cb[nat] * C1 + ranks[nat]
        nkv_idx[c, cg[nat], slots] = cl[nat]
        q1_idx[c, cg[nat], slots] = cb[nat]
        offs1[c, cg[nat], slots] = 0.0
        # relocation of overflow pairs
        occ = np.zeros((NG, B), dtype=np.int32)
        np.add.at(occ, (cg[nat], cb[nat]), 1)
        ov_i = np.nonzero(~nat)[0]
        for i in ov_i:
            b_, e_ = int(cb[i]), int(ce[i])
            g2 = int(np.argmin(occ[:, b_]))
            if occ[g2, b_] >= C1:
                rng_fail.append(("ph1_capacity", c, b_))
                continue
            r2 = occ[g2, b_]
            occ[g2, b_] += 1
            emap = ext_maps[c * NG + g2]
            if e_ in emap:
                pos = emap[e_]
            else:
                pos = len(emap)
                if pos >= EXT1:
                    rng_fail.append(("ph1_ext", c, g2))
                    continue
                emap[e_] = pos
                nei_ext[c, g2, pos] = nei8[e_]
            s2 = b_ * C1 + r2
            nkv_idx[c, g2, s2] = EPB + pos
            q1_idx[c, g2, s2] = b_
            offs1[c, g2, s2] = 0.0

    if rng_fail:
        raise RuntimeError(f"capacity exceeded: {rng_fail[:5]}")

    # ---------------- phase-2 routing ------------------------------------
    e2 = dst.reshape(-1).astype(np.int64)        # [B*N]
    b2 = np.repeat(np.arange(B, dtype=np.int64), N)
    bkt2 = e2 // EPB
    core2 = bkt2 // NG
    grp2 = bkt2 % NG
    loc2 = e2 - bkt2 * EPB
    order2 = np.argsort(bkt2, kind="stable")
    counts = np.bincount(bkt2, minlength=NBK)
    if counts.max() > L2:
        raise RuntimeError(f"phase2 bucket overflow: {counts.max()} > {L2}")
    slot_in_bkt = np.empty(B * N, dtype=np.int64)
    sorted_ranks = (np.arange(B * N)
                    - np.repeat(np.concatenate(([0], np.cumsum(counts)[:-1])),
                                counts))
    slot_in_bkt[order2] = sorted_ranks

    ent_idx = np.zeros((NCORES, NG, L2), dtype=np.int64)
    t_idx = np.zeros((NCORES, NG, L2), dtype=np.int64)
    ent_idx[core2, grp2, slot_in_bkt] = loc2
    t_idx[core2, grp2, slot_in_bkt] = b2
    # host scatter map: flat pair index -> (core, grp*L2 + slot)
    out_pos = (core2, grp2 * L2 + slot_in_bkt)

    in_maps = []
    for c in range(NCORES):
        ent_c = ent_bkt[c * NG:(c + 1) * NG]                    # [8,EPB,16,16]
        ent_tbl = (ent_c.transpose(0, 2, 1, 3)
                   .reshape(128, TE, 16))
        nei_c = nei_bkt[c * NG:(c + 1) * NG]
        next_c = nei_ext[c].reshape(NG, EXT1, 16, 16)
        nei_full = np.concatenate([nei_c, next_c], axis=1)      # [8,TN,16,16]
        nei_tbl = (nei_full.transpose(0, 2, 1, 3)
                   .reshape(128, TN, 16))
        in_maps.append({
            "ent_tbl": np.ascontiguousarray(ent_tbl),
            "nei_tbl": np.ascontiguousarray(nei_tbl),
            "qt": np.ascontiguousarray(qt_il),
            "base": np.ascontiguousarray(base_il),
            "wrep": wrep,
            "nkv_idx": _pack_idx(nkv_idx[c]),
            "ent_idx": _pack_idx(ent_idx[c]),
            "t_idx": _pack_idx(t_idx[c]),
            "offs1": offs1[c].reshape(NG, L1)[:8].astype(np.float32),
            "w8": w8, "w2": w2, "wc": wc,
        })
    return in_maps, out_pos


def kernel(src, rel, dst, mode, ent_embed, rel_embed, nei_embed, weight_embed,
           neiMatrix):
    global LAST_RESULT
    if int(mode) != 0:
        raise NotImplementedError("only mode==0 (tail_batch) is supported")

    src = np.asarray(src, dtype=np.int64)
    rel = np.asarray(rel, dtype=np.int64)
    dst = np.asarray(dst, dtype=np.int64)
    ent_embed = np.ascontiguousarray(np.asarray(ent_embed, dtype=np.float32))
    rel_embed = np.ascontiguousarray(np.asarray(rel_embed, dtype=np.float32))
    nei_embed = np.ascontiguousarray(np.asarray(nei_embed, dtype=np.float32))
    weight_embed = np.asarray(weight_embed, dtype=np.float32)
    neiMatrix = np.asarray(neiMatrix, dtype=np.int64)

    nc = _get_program()
    in_maps, out_pos = make_in_maps(src, rel, dst, ent_embed, rel_embed,
                                    nei_embed, weight_embed, neiMatrix)
    res = run_bass_kernel_spmd(nc, in_maps, list(range(NCORES)))
    LAST_RESULT = res
    outs = np.stack([np.asarray(res.results[c]["out"]).reshape(-1)
                     for c in range(NCORES)], axis=0)      # [8, 8*L2]
    score_flat = -outs[out_pos[0], out_pos[1]]
    return score_flat.reshape(B, N).astype(np.float32)


# revision 24
# speedup vs baseline: 1.0610x; 1.0610x over previous
"""CoNE KG-embedding scoring kernel for 8 Trainium2 NeuronCores.

Strategy (v2): entity-sharded SBUF-resident fp8 tables + GPSIMD ap_gather.

The two big gathers (nei_embed[neiMatrix[src]] and ent_embed[dst]) are the
memory-bound core of this problem.  indirect DMA runs ~139ns/row on one
queue; instead we shard both tables into 64 = 8 cores x 8 (16-partition
groups) buckets of EPB=1563 entities.  Partition 16g+i of core c holds
d-slice [16i,16i+16) of bucket (c,g)'s entities, fp8.  A single ap_gather
instruction then gathers, per group, an independent per-group list of rows
(~30ns/row, 8 Q7 cores in parallel).

Each (b,n) / (b,k) pair is routed on host to the bucket owning its entity.
Phase 1 (neighbor attention) accumulates unnormalized softmax numerators
v[b] = sum_k exp(q.nkv/16) * nkv and l[b] = sum_k exp(.) in a fixed layout
of C1=3 slots per (group, b) (overflow pairs are relocated to other groups,
with the entity row appended to that group's table extension), reduces
per-b on DVE, combines groups on PE, and AllReduces across cores.
t = (1-w)*ie + rel + w*v/l is formed directly in the gather ("tilde")
layout: q~ and base~ = (1-w)*ie + rel are host-precomputed uploads.
Phase 2 gathers ent rows and t rows per pair and reduces |t - pe| on
DVE + PE (block-ones matmul) into per-(group, slot) scores; the host
scatters them back to [B, N] and negates.
"""

import numpy as np
import ml_dtypes

import concourse.bacc as bacc
import concourse.bass as bass
import concourse.mybir as mybir
import concourse.tile as tile
from concourse.bass_utils import run_bass_kernel_spmd

F32 = mybir.dt.float32
BF16 = mybir.dt.bfloat16
FP8 = mybir.dt.float8e4
I16 = mybir.dt.int16
NP_FP8 = ml_dtypes.float8_e4m3
NP_BF16 = ml_dtypes.bfloat16

E, R, D, K, B, N = 100000, 500, 256, 64, 1024, 256
NCORES = 8
NG = 8                 # 16-partition groups per core
NBK = NCORES * NG      # 64 entity buckets
EPB = 1563             # entities per bucket (64*1563 = 100032 >= E)
TE = EPB               # ent table rows per group
EXT1 = 192             # nei table extension rows (relocated entities)
TN = EPB + EXT1
C1 = 3                 # phase-1 slots per (group, b)
L1 = C1 * B            # 3072 phase-1 slots per group
CH1 = 384              # phase-1 chunk (128 b's)
L2 = 4608              # phase-2 slots per group (mean 4096, +8 sigma)
CH2 = 384              # phase-2 chunk
MCOL = 384             # matmul free-dim chunk (psum-bank safe)
MASK_OFF = -1.0e9
INV_SQRT_D = 1.0 / 16.0

_PROGRAMS = {}
LAST_RESULT = None


def _build_program(iters=1, skip_ph1=False, skip_ph2=False, skip_ar=False, dbl_g2=False, ph2_nocompute=False):
    nc = bacc.Bacc(
        "TRN2",
        target_bir_lowering=False,
        debug=False,
        enable_asserts=False,
        num_devices=NCORES,
    )

    def din(name, shape, dt):
        return nc.dram_tensor(name, shape, dt, kind="ExternalInput").ap()

    ent_t = din("ent_tbl", [128, TE, 16], FP8)
    nei_t = din("nei_tbl", [128, TN, 16], FP8)
    qt_t = din("qt", [128, B, 16], FP8)
    base_t = din("base", [128, B, 16], FP8)
    wrep_t = din("wrep", [128, B], BF16)
    nkv_i = din("nkv_idx", [128, L1 // 16], I16)
    ent_i = din("ent_idx", [128, L2 // 16], I16)
    t_i = din("t_idx", [128, L2 // 16], I16)
    offs_t = din("offs1", [8, L1], F32)
    w8_t = din("w8", [128, 8], BF16)
    w2_t = din("w2", [8, 128], BF16)
    wc_t = din("wc", [128, 16], BF16)
    out_t = nc.dram_tensor("out", [8, L2], F32, kind="ExternalOutput").ap()

    ar_in = [
        nc.dram_tensor(f"ar_in{it}", [16, B * 18], BF16, kind="Internal").ap()
        for it in range(iters)
    ]
    ar_out = [
        nc.dram_tensor(f"ar_out{it}", [16, B * 18], BF16, kind="Internal",
                       addr_space="Shared").ap()
        for it in range(iters)
    ]
    groups = [list(range(NCORES))]

    with tile.TileContext(nc) as tc:
      with (
          tc.tile_pool(name="persist", bufs=1) as pp,
          tc.tile_pool(name="pent", bufs=4) as p2e,
          tc.psum_pool(name="ps", bufs=2) as psp,
      ):
        for it in range(iters):
            # ---------- persistent small loads ---------------------------
            w8 = pp.tile([128, 8], BF16, tag="w8")
            nc.sync.dma_start(out=w8[:], in_=w8_t[:])
            w2 = pp.tile([8, 128], BF16, tag="w2")
            nc.sync.dma_start(out=w2[:], in_=w2_t[:])
            wc = pp.tile([128, 16], BF16, tag="wc")
            nc.sync.dma_start(out=wc[:], in_=wc_t[:])
            vt = pp.tile([128, B, 18], BF16, tag="vt")
            tt = pp.tile([128, B, 16], FP8, tag="tt")
            ent_sb = pp.tile([128, TE, 16], FP8, tag="ent")
            nc.sync.dma_start(out=ent_sb[:], in_=ent_t[:])
            ent_idx = pp.tile([128, L2 // 16], I16, tag="enti")
            nc.sync.dma_start(out=ent_idx[:], in_=ent_i[:])
            ent_gs = {}

            def gather_ent(ch):
                csl_ = slice(ch * (CH2 // 16), (ch + 1) * (CH2 // 16))
                eg = p2e.tile([128, CH2, 16], FP8, tag="eg")
                for _r in range(2 if dbl_g2 else 1):
                    nc.gpsimd.ap_gather(eg[:], ent_sb[:],
                                        ent_idx[:, csl_], 128, TE, 16, CH2)
                ent_gs[ch] = eg

            # ---------- phase 1a: sharded neighbor attention -------------
            with tc.tile_pool(name="ph1g", bufs=2) as p1g, \
                 tc.tile_pool(name="ph1d", bufs=2) as p1d, \
                 tc.tile_pool(name="ph1a", bufs=1) as p1a:
                nei_sb = p1a.tile([128, TN, 16], FP8, tag="nei")
                nc.sync.dma_start(out=nei_sb[:], in_=nei_t[:])
                qt = p1a.tile([128, B, 16], FP8, tag="qt")
                nc.sync.dma_start(out=qt[:], in_=qt_t[:])
                nkv_idx = p1a.tile([128, L1 // 16], I16, tag="nkvi")
                nc.sync.dma_start(out=nkv_idx[:], in_=nkv_i[:])
                offs = p1a.tile([8, L1], F32, tag="offs")
                nc.sync.dma_start(out=offs[:], in_=offs_t[:])

                if skip_ph1:
                    nc.vector.memset(vt[:], 1.0)
                for ch in range(0 if skip_ph1 else L1 // CH1):
                    csl = slice(ch * (CH1 // 16), (ch + 1) * (CH1 // 16))
                    nkv_g = p1g.tile([128, CH1, 16], FP8, tag="nkv")
                    nc.gpsimd.ap_gather(nkv_g[:], nei_sb[:],
                                        nkv_idx[:, csl], 128, TN, 16, CH1)
                    # scores: per-partition partial dot, then group-sum on PE
                    # q side needs no gather: slot = b*C1 + c, so q~[b] is a
                    # stride-0 broadcast over the C1 slot repeats.
                    bsl0 = slice(ch * (CH1 // C1), (ch + 1) * (CH1 // C1))
                    q_bc = (qt[:, bsl0, None, :]
                            .to_broadcast([128, CH1 // C1, C1, 16]))
                    prod = p1d.tile([128, CH1, 18], BF16, tag="prod")
                    sprod = prod[:, :, 0:16]
                    nc.vector.tensor_mul(
                        out=sprod.rearrange("p (b c) j -> p b c j", c=C1),
                        in0=nkv_g[:].rearrange("p (b c) j -> p b c j", c=C1),
                        in1=q_bc)
                    sred = p1d.tile([128, CH1], BF16, tag="sred")
                    with nc.allow_low_precision(reason="16-term bf16 sum"):
                        nc.vector.tensor_reduce(
                            out=sred[:], in_=sprod, axis=mybir.AxisListType.X,
                            op=mybir.AluOpType.add)
                    p_sb = p1d.tile([8, CH1], BF16, tag="p")
                    for m in range(CH1 // MCOL):
                        msl = slice(m * MCOL, (m + 1) * MCOL)
                        ps = psp.tile([8, MCOL], F32, tag="sc")
                        nc.tensor.matmul(ps[:], w8[:], sred[:, msl],
                                         start=True, stop=True)
                        sc2 = p1d.tile([8, MCOL], F32, tag="sc2")
                        nc.vector.tensor_add(
                            out=sc2[:], in0=ps[:],
                            in1=offs[:, ch * CH1 + m * MCOL:
                                     ch * CH1 + (m + 1) * MCOL])
                        nc.scalar.activation(
                            out=p_sb[:, msl], in_=sc2[:],
                            func=mybir.ActivationFunctionType.Exp,
                            scale=INV_SQRT_D)
                    # broadcast p to all 16 partitions of each group
                    pb = p1d.tile([128, CH1], F32, tag="pb")
                    for m in range(CH1 // MCOL):
                        msl = slice(m * MCOL, (m + 1) * MCOL)
                        psb = psp.tile([128, MCOL], F32, tag="pb_ps")
                        nc.tensor.matmul(psb[:], w2[:], p_sb[:, msl],
                                         start=True, stop=True)
                        nc.vector.tensor_copy(out=pb[:, msl], in_=psb[:])
                    nc.vector.tensor_mul(
                        out=prod[:, :, 0:16], in0=nkv_g[:],
                        in1=pb[:, :, None].to_broadcast([128, CH1, 16]))
                    nc.vector.tensor_copy(
                        out=prod[:, :, 16:18],
                        in_=pb[:, :, None].to_broadcast([128, CH1, 2]))
                    # segment-reduce the C1 slots per b directly into vt
                    bsl = slice(ch * (CH1 // C1), (ch + 1) * (CH1 // C1))
                    with nc.allow_low_precision(reason="3-term bf16 sum"):
                        nc.vector.tensor_reduce(
                            out=vt[:, bsl, :],
                            in_=prod[:].rearrange("p (b c) j -> p b j c", c=C1),
                            axis=mybir.AxisListType.X,
                            op=mybir.AluOpType.add)

            # prefetch phase-2 ent-side gathers; they run on the Q7 while
            # the DVE/PE finish phase 1 and the AllReduce is in flight
            PREF = 4
            NCH2 = 0 if skip_ph2 else L2 // CH2
            for ch in range(min(PREF, NCH2)):
                gather_ent(ch)

            # ---------- phase 1b: combine groups, AllReduce, build t~ ----
            with tc.tile_pool(name="ph1b", bufs=1) as p1b:
                stage = p1b.tile([16, B * 18], BF16, tag="stage")
                vt_flat = vt[:].rearrange("p b j -> p (b j)")
                for m in range((B * 18) // MCOL):
                    msl = slice(m * MCOL, (m + 1) * MCOL)
                    psc = psp.tile([16, MCOL], F32, tag="vc")
                    nc.tensor.matmul(psc[:], wc[:], vt_flat[:, msl],
                                     start=True, stop=True)
                    nc.vector.tensor_copy(out=stage[:, msl], in_=psc[:])
                nc.sync.dma_start(out=ar_in[it][:], in_=stage[:])
                if not skip_ar:
                    nc.gpsimd.collective_compute(
                        kind="AllReduce", op=mybir.AluOpType.add,
                        replica_groups=groups, ins=[ar_in[it][:]],
                        outs=[ar_out[it][:]])
                else:
                    nc.sync.dma_start(out=ar_out[it][:], in_=ar_in[it][:])
                # load the reduced [16, B*18] into group 0, cast to bf16,
                # then replicate to the other 7 groups
                vall = p1b.tile([128, B, 18], BF16, tag="vall")
                vall_f = vall[:].rearrange("p b j -> p (b j)")
                nc.sync.dma_start(out=vall_f[0:16, :], in_=ar_out[it][:])
                for g in range(1, NG):
                    nc.sync.dma_start(
                        out=vall_f[g * 16:(g + 1) * 16, :], in_=vall_f[0:16, :])
                # t~ = base + wrep * v/l   (all in gather layout)
                wrep = p1b.tile([128, B], BF16, tag="wrep")
                nc.sync.dma_start(out=wrep[:], in_=wrep_t[:])
                base_sb = p1b.tile([128, B, 16], FP8, tag="base")
                nc.sync.dma_start(out=base_sb[:], in_=base_t[:])
                with nc.allow_low_precision(reason="elementwise, not accum"):
                    nc.vector.reciprocal(out=vall[:, :, 17],
                                         in_=vall[:, :, 16])
                    nc.vector.tensor_mul(
                        out=vall[:, :, 0:16], in0=vall[:, :, 0:16],
                        in1=vall[:, :, 17:18].to_broadcast([128, B, 16]))
                    nc.vector.tensor_mul(
                        out=vall[:, :, 0:16], in0=vall[:, :, 0:16],
                        in1=wrep[:, :, None].to_broadcast([128, B, 16]))
                    nc.vector.tensor_add(out=tt[:], in0=vall[:, :, 0:16],
                                         in1=base_sb[:])

            # ---------- phase 2: TransE-L1 scores ------------------------
            with tc.tile_pool(name="ph2g", bufs=2) as p2g, \
                 tc.tile_pool(name="ph2d", bufs=2) as p2d, \
                 tc.tile_pool(name="ph2c", bufs=1) as p2c:
                t_idx = p2c.tile([128, L2 // 16], I16, tag="ti")
                nc.sync.dma_start(out=t_idx[:], in_=t_i[:])
                scores = p2c.tile([8, L2], F32, tag="scores")
                if (ph2_nocompute or skip_ph2):
                    nc.vector.memset(scores[:], 0.0)
                for ch in range(NCH2):
                    csl = slice(ch * (CH2 // 16), (ch + 1) * (CH2 // 16))
                    t_g = p2g.tile([128, CH2, 16], FP8, tag="tg")
                    for _r in range(2 if dbl_g2 else 1):
                        nc.gpsimd.ap_gather(t_g[:], tt[:],
                                            t_idx[:, csl], 128, B, 16, CH2)
                    if ch + PREF < NCH2:
                        gather_ent(ch + PREF)
                    ent_g = ent_gs.pop(ch)
                    if ph2_nocompute:
                        continue
                    # |a-b| = max(a,b) - min(a,b), all on DVE (no engine hops)
                    df = p2d.tile([128, CH2, 16], BF16, tag="df")
                    mn = p2d.tile([128, CH2, 16], BF16, tag="mn")
                    nc.vector.tensor_tensor(out=df[:], in0=ent_g[:],
                                            in1=t_g[:],
                                            op=mybir.AluOpType.max)
                    nc.vector.tensor_tensor(out=mn[:], in0=ent_g[:],
                                            in1=t_g[:],
                                            op=mybir.AluOpType.min)
                    nc.vector.tensor_sub(out=df[:], in0=df[:], in1=mn[:])
                    dred = p2d.tile([128, CH2], BF16, tag="dred")
                    with nc.allow_low_precision(reason="16-term bf16 sum"):
                        nc.vector.tensor_reduce(
                            out=dred[:], in_=df[:], axis=mybir.AxisListType.X,
                            op=mybir.AluOpType.add)
                    for m in range(CH2 // MCOL):
                        msl = slice(m * MCOL, (m + 1) * MCOL)
                        ps2 = psp.tile([8, MCOL], F32, tag="sc2b")
                        nc.tensor.matmul(ps2[:], w8[:], dred[:, msl],
                                         start=True, stop=True)
                        nc.vector.tensor_copy(
                            out=scores[:, ch * CH2 + m * MCOL:
                                       ch * CH2 + (m + 1) * MCOL],
                            in_=ps2[:])
                nc.sync.dma_start(out=out_t[:], in_=scores[:])

    nc.compile()
    return nc


def _get_program(iters=1):
    if iters not in _PROGRAMS:
        _PROGRAMS[iters] = _build_program(iters)
    return _PROGRAMS[iters]


# ---------------------------------------------------------------------------
# host-side preparation
# ---------------------------------------------------------------------------

def _interleave(x):
    """[B, 256] -> [128, B, 16] tilde layout: out[16g+i, b, j] = x[b, 16i+j]."""
    b = x.shape[0]
    base = x.reshape(b, 16, 16).transpose(1, 0, 2)          # [i, b, j]
    return np.tile(base, (NG, 1, 1))                        # [128, b, 16]


def _pack_idx(lists):
    """[NG, L] int16 -> wrapped [128, L//16]: arr[16g+p, c] = lists[g][c*16+p]."""
    ng, L = lists.shape
    return (lists.reshape(ng, L // 16, 16)
            .transpose(0, 2, 1)
            .reshape(ng * 16, L // 16)
            .astype(np.int16))


def make_in_maps(src, rel, dst, ent_embed, rel_embed, nei_embed, weight_embed,
                 neiMatrix):
    rng_fail = []

    ent8 = np.zeros((NBK * EPB, D), dtype=NP_FP8)
    ent8[:E] = ent_embed.astype(NP_FP8)
    nei8 = np.zeros((NBK * EPB, D), dtype=NP_FP8)
    nei8[:E] = nei_embed.astype(NP_FP8)

    # d16 bucketed tables: [64, EPB, 16i, 16j] -> per core [8g,16i,EPB,16j]
    ent_bkt = ent8.reshape(NBK, EPB, 16, 16)
    nei_bkt = nei8.reshape(NBK, EPB, 16, 16)

    ie = ent_embed[src]                          # [B, D] f32
    rel_e = rel_embed[rel]                       # [B, D]
    w = 1.0 / (1.0 + np.exp(-weight_embed[src].reshape(B, 1)))  # sigmoid
    q = ie + rel_e
    base = (1.0 - w) * ie + rel_e

    qt_il = _interleave(q.astype(NP_FP8))                    # [128, B, 16] fp8
    base_il = _interleave(base.astype(NP_FP8))               # [128, B, 16] fp8
    wrep = np.tile(w.reshape(1, B), (128, 1)).astype(NP_BF16)

    w8 = np.zeros((128, 8), dtype=NP_BF16)
    w8[np.arange(128), np.arange(128) // 16] = 1.0
    w2 = np.zeros((8, 128), dtype=NP_BF16)
    w2[np.arange(128) // 16, np.arange(128)] = 1.0
    wc = np.zeros((128, 16), dtype=NP_BF16)
    wc[np.arange(128), np.arange(128) % 16] = 1.0

    # ---------------- phase-1 routing (per core) -------------------------
    nei_ids = neiMatrix[src]                     # [B, K]
    valid = nei_ids > 0
    bb, kk = np.nonzero(valid)
    ee = nei_ids[bb, kk]
    bkt = ee // EPB
    core1 = bkt // NG
    grp1 = bkt % NG
    loc1 = ee - bkt * EPB

    nkv_idx = np.zeros((NCORES, NG, L1), dtype=np.int64)
    q1_idx = np.zeros((NCORES, NG, L1), dtype=np.int64)
    offs1 = np.full((NCORES, NG, L1), MASK_OFF, dtype=np.float32)
    ext_maps = [dict() for _ in range(NCORES * NG)]   # (c,g) -> {e: extpos}
    nei_ext = np.zeros((NCORES, NG, EXT1, D), dtype=NP_FP8)

    for c in range(NCORES):
        m = core1 == c
        cb, cg, cl, ce = bb[m], grp1[m], loc1[m], ee[m]
        # rank within (g, b)
        order = np.lexsort((cb, cg))
        cb, cg, cl, ce = cb[order], cg[order], cl[order], ce[order]
        grp_key = cg * B + cb
        # cumcount per key
        uniq, starts = np.unique(grp_key, return_index=True)
        ranks = np.arange(len(grp_key))
        ranks = ranks - np.repeat(starts, np.diff(np.append(starts,
                                                            len(grp_key))))
        nat = ranks < C1
        slots = cb[nat] * C1 + ranks[nat]
        nkv_idx[c, cg[nat], slots] = cl[nat]
        q1_idx[c, cg[nat], slots] = cb[nat]
        offs1[c, cg[nat], slots] = 0.0
        # relocation of overflow pairs
        occ = np.zeros((NG, B), dtype=np.int32)
        np.add.at(occ, (cg[nat], cb[nat]), 1)
        ov_i = np.nonzero(~nat)[0]
        for i in ov_i:
            b_, e_ = int(cb[i]), int(ce[i])
            g2 = int(np.argmin(occ[:, b_]))
            if occ[g2, b_] >= C1:
                rng_fail.append(("ph1_capacity", c, b_))
                continue
            r2 = occ[g2, b_]
            occ[g2, b_] += 1
            emap = ext_maps[c * NG + g2]
            if e_ in emap:
                pos = emap[e_]
            else:
                pos = len(emap)
                if pos >= EXT1:
                    rng_fail.append(("ph1_ext", c, g2))
                    continue
                emap[e_] = pos
                nei_ext[c, g2, pos] = nei8[e_]
            s2 = b_ * C1 + r2
            nkv_idx[c, g2, s2] = EPB + pos
            q1_idx[c, g2, s2] = b_
            offs1[c, g2, s2] = 0.0

    if rng_fail:
        raise RuntimeError(f"capacity exceeded: {rng_fail[:5]}")

    # ---------------- phase-2 routing ------------------------------------
    e2 = dst.reshape(-1).astype(np.int64)        # [B*N]
    b2 = np.repeat(np.arange(B, dtype=np.int64), N)
    bkt2 = e2 // EPB
    core2 = bkt2 // NG
    grp2 = bkt2 % NG
    loc2 = e2 - bkt2 * EPB
    order2 = np.argsort(bkt2, kind="stable")
    counts = np.bincount(bkt2, minlength=NBK)
    if counts.max() > L2:
        raise RuntimeError(f"phase2 bucket overflow: {counts.max()} > {L2}")
    slot_in_bkt = np.empty(B * N, dtype=np.int64)
    sorted_ranks = (np.arange(B * N)
                    - np.repeat(np.concatenate(([0], np.cumsum(counts)[:-1])),
                                counts))
    slot_in_bkt[order2] = sorted_ranks

    ent_idx = np.zeros((NCORES, NG, L2), dtype=np.int64)
    t_idx = np.zeros((NCORES, NG, L2), dtype=np.int64)
    ent_idx[core2, grp2, slot_in_bkt] = loc2
    t_idx[core2, grp2, slot_in_bkt] = b2
    # host scatter map: flat pair index -> (core, grp*L2 + slot)
    out_pos = (core2, grp2 * L2 + slot_in_bkt)

    in_maps = []
    for c in range(NCORES):
        ent_c = ent_bkt[c * NG:(c + 1) * NG]                    # [8,EPB,16,16]
        ent_tbl = (ent_c.transpose(0, 2, 1, 3)
                   .reshape(128, TE, 16))
        nei_c = nei_bkt[c * NG:(c + 1) * NG]
        next_c = nei_ext[c].reshape(NG, EXT1, 16, 16)
        nei_full = np.concatenate([nei_c, next_c], axis=1)      # [8,TN,16,16]
        nei_tbl = (nei_full.transpose(0, 2, 1, 3)
                   .reshape(128, TN, 16))
        in_maps.append({
            "ent_tbl": np.ascontiguousarray(ent_tbl),
            "nei_tbl": np.ascontiguousarray(nei_tbl),
            "qt": np.ascontiguousarray(qt_il),
            "base": np.ascontiguousarray(base_il),
            "wrep": wrep,
            "nkv_idx": _pack_idx(nkv_idx[c]),
            "ent_idx": _pack_idx(ent_idx[c]),
            "t_idx": _pack_idx(t_idx[c]),
            "offs1": offs1[c].reshape(NG, L1)[:8].astype(np.float32),
            "w8": w8, "w2": w2, "wc": wc,
        })
    return in_maps, out_pos


def kernel(src, rel, dst, mode, ent_embed, rel_embed, nei_embed, weight_embed,
           neiMatrix):
    global LAST_RESULT
    if int(mode) != 0:
        raise NotImplementedError("only mode==0 (tail_batch) is supported")

    src = np.asarray(src, dtype=np.int64)
    rel = np.asarray(rel, dtype=np.int64)
    dst = np.asarray(dst, dtype=np.int64)
    ent_embed = np.ascontiguousarray(np.asarray(ent_embed, dtype=np.float32))
    rel_embed = np.ascontiguousarray(np.asarray(rel_embed, dtype=np.float32))
    nei_embed = np.ascontiguousarray(np.asarray(nei_embed, dtype=np.float32))
    weight_embed = np.asarray(weight_embed, dtype=np.float32)
    neiMatrix = np.asarray(neiMatrix, dtype=np.int64)

    nc = _get_program()
    in_maps, out_pos = make_in_maps(src, rel, dst, ent_embed, rel_embed,
                                    nei_embed, weight_embed, neiMatrix)
    res = run_bass_kernel_spmd(nc, in_maps, list(range(NCORES)))
    LAST_RESULT = res
    outs = np.stack([np.asarray(res.results[c]["out"]).reshape(-1)
                     for c in range(NCORES)], axis=0)      # [8, 8*L2]
    score_flat = -outs[out_pos[0], out_pos[1]]
    return score_flat.reshape(B, N).astype(np.float32)
